# revision 31
# baseline (speedup 1.0000x reference)
"""3x3 valid conv (cross-correlation) + bias on a 4096x4096 fp32 image,
run across 8 trn2 NeuronCores.

Strategy
--------
Rows are sharded across the 8 cores host-side with a 2-row halo folded
into each core's input slice (no device collectives needed). On each
core the conv is computed as banded matmuls on the TensorEngine:

  For an output row-tile of M=126 rows (input rows K=M+2), and each of
  the 3 kernel columns dj, build a banded stationary matrix
  B_dj[k, m] = w[k-m, dj] (zero outside 0<=k-m<=2). Then

      Y_tile[m, n] = sum_dj sum_k B_dj[k, m] * X_tile[k, n+dj]

  i.e. matmuls accumulating in PSUM per 512-wide column chunk, with
  the dj shift expressed in the rhs access pattern. Bias is added during
  the PSUM->SBUF copy (DMA cannot read PSUM) on DVE/ACT in parallel.

Startup latency is dominated by DMA descriptor-rate, not bytes, so the
two weight-band tensors load on the Scalar HWDGE queue while the image
tiles load on the Sync queue; stores are split across both queues.

Variants (VARIANT):
  f32  - exact fp32 matmuls (4 cycles/row on the PE: slowest)
  f32r - TF32-like fp32r matmuls (1 cycle/row, ~2e-4 rel err)
  f16  - host-casts X and the bands to fp16 (halves input DMA,
         1 cycle/row, ~1.5e-4 rel err), fp32 output
  f16o - like f16 but the output is written as fp16 too (halves the
         store DMA; ~5e-4 rel err from output quantization)
  f16c - compensated fp16: X = Xh + Xl, W = Wh + Wl (both splits exact
         to ~2^-22), Y = Wh@Xh + Wl@Xh + Wh@Xl. 9 matmuls/chunk but
         ~5e-7 rel err at 1 cycle/row.
"""

import numpy as np

H = 4096
W = 4096
KH = 3
KW = 3
HOUT = H - KH + 1  # 4094
WOUT = W - KW + 1  # 4094
NCORES = 8
ROWS_PER_CORE = 512          # output rows computed per core
IN_ROWS = ROWS_PER_CORE + 2  # input rows per core (with halo)
# Core 7 overlaps core 6 by 2 rows so that all shards have equal shape.
STARTS = [0, 512, 1024, 1536, 2048, 2560, 3072, 3582]
M_TILE = 126                 # output rows per matmul tile (K = M + 2 <= 128)
N_CHUNK = 512                # PSUM bank = 512 fp32

# tile-0 input column halves (half B starts early so chunk 4's
# dj-shifted reads fit)
XA_LO, XA_W = 0, 2052
XB_LO, XB_W = 2046, 2050

# remainder strip: last R_STRIP output rows, packed as NSEG column
# segments stacked across partitions (NSEG*STRIP_IN partitions)
N_FULL_TILES = ROWS_PER_CORE // M_TILE   # 4
R_STRIP = ROWS_PER_CORE - N_FULL_TILES * M_TILE  # 8
STRIP_IN = R_STRIP + 2                   # 10
NSEG = 8
SEG = W // NSEG                          # 512

VARIANT = "f16o"

_PROGRAM_CACHE = {}


def _build_program(variant: str):
    import concourse.mybir as mybir
    from concourse import bacc
    from concourse.tile import TileContext

    f32 = mybir.dt.float32
    mm_dt = {
        "f32": f32,
        "f32r": mybir.dt.float32r,
        "f16": mybir.dt.float16,
        "f16o": mybir.dt.float16,
        "f16c": mybir.dt.float16,
    }[variant]
    out_dt = mybir.dt.float16 if variant == "f16o" else f32

    nc = bacc.Bacc()
    # x inputs: one per term-split (f16c needs hi and lo parts)
    nxparts = 2 if variant == "f16c" else 1
    nwparts = 2 if variant == "f16c" else 1
    xs = [
        nc.declare_dram_parameter(f"x{i}", [IN_ROWS, W], mm_dt, isOutput=False)
        for i in range(nxparts)
    ]
    # bands: full-tile planes, then strip planes, then one trailing bias
    # column — a single DMA (descriptor count, not bytes, gates startup)
    NBT = KW * nwparts * 128
    bands = nc.declare_dram_parameter(
        "bands", [128, 2 * NBT + 1], mm_dt, isOutput=False
    )
    # host-packed remainder strip: NSEG column segments of the last
    # STRIP_IN input rows stacked across partitions, with 2-col halos
    ss = [
        nc.declare_dram_parameter(
            f"s{i}", [NSEG * STRIP_IN, SEG + 2], mm_dt, isOutput=False
        )
        for i in range(nxparts)
    ]
    y = nc.declare_dram_parameter("y", [ROWS_PER_CORE, WOUT], out_dt, isOutput=True)
    ysd = nc.declare_dram_parameter(
        "ys", [NSEG * R_STRIP, SEG], out_dt, isOutput=True
    )

    n_tiles = N_FULL_TILES
    chunks = []
    n0 = 0
    while n0 < WOUT:
        chunks.append((n0, min(N_CHUNK, WOUT - n0)))
        n0 += N_CHUNK
    NGRP = 4                      # chunks per PSUM group (4 banks)

    # (weight part index, x part index) per accumulation term:
    # f16c: Wh@Xh + Wl@Xh + Wh@Xl
    terms = [(0, 0), (1, 0), (0, 1)] if variant == "f16c" else [(0, 0)]

    npart = NSEG * STRIP_IN           # 80
    nout = NSEG * R_STRIP             # 64

    with TileContext(nc) as tc:
        with (
            tc.tile_pool(name="consts", bufs=1) as consts,
            tc.tile_pool(name="xp", bufs=n_tiles) as xp,
            tc.tile_pool(name="sp", bufs=1) as sp,
            tc.tile_pool(name="yp", bufs=2) as yp,
            tc.tile_pool(name="pp", bufs=2, space="PSUM") as pp,
        ):
            # All loads go on the Sync HWDGE queue in need order (a second
            # parallel queue measurably slows the first completions). The
            # queue's first transfers are descriptor-rate-bound, so the
            # critical prefix is: bands, strip, tile-0 half A.
            bands_sb = consts.tile([128, 2 * NBT + 1], mm_dt)
            nc.sync.dma_start(out=bands_sb[:], in_=bands[:])
            bandT = bands_sb[:, :NBT].rearrange("p (a b) -> p a b", b=128)
            bandS = bands_sb[:, NBT : 2 * NBT].rearrange("p (a b) -> p a b", b=128)
            # upcast the fp16 bias column once on the (otherwise idle)
            # GPSIMD engine; DVE/ACT need an fp32 scalar operand
            bias_sb = consts.tile([128, 1], f32)
            nc.gpsimd.tensor_copy(bias_sb[:, :], bands_sb[:, 2 * NBT :])

            # PE p-state warmup tile: the Tensor engine ramps 0.65 -> 1.2
            # -> 2.4 GHz only after ~3us of continuous execution. Dummy
            # matmuls on this gpsimd-zeroed tile (no DMA dependency) run
            # while the first input loads are in flight, so the real
            # stream starts at full clock.
            warm = consts.tile([128, N_CHUNK], mm_dt)
            nc.gpsimd.memset(warm[:, :], 0)

            # image loads: tile 0 in two column halves (group 0 only waits
            # for half A), tiles 1..3 full-width
            k0 = M_TILE + KH - 1
            xa0, xb0 = [], []
            for i in range(nxparts):
                ta = xp.tile([128, XA_W], mm_dt, tag=f"xa{i}")
                nc.sync.dma_start(
                    out=ta[:k0, :], in_=xs[i][0:k0, XA_LO : XA_LO + XA_W]
                )
                xa0.append(ta)
            for i in range(nxparts):
                tb = xp.tile([128, XB_W], mm_dt, tag=f"xb{i}")
                nc.sync.dma_start(
                    out=tb[:k0, :], in_=xs[i][0:k0, XB_LO : XB_LO + XB_W]
                )
                xb0.append(tb)
            strips = []
            for i in range(nxparts):
                st = sp.tile([npart, SEG + 2], mm_dt, tag=f"strip{i}")
                nc.sync.dma_start(out=st[:, :], in_=ss[i][:, :])
                strips.append(st)
            xts = [None]
            for t in range(1, n_tiles):
                r0 = t * M_TILE
                k = min(M_TILE, ROWS_PER_CORE - r0) + KH - 1
                parts = []
                for i in range(nxparts):
                    xt = xp.tile([128, W], mm_dt, tag=f"x{i}")
                    nc.sync.dma_start(out=xt[:k, :], in_=xs[i][r0 : r0 + k, :])
                    parts.append(xt)
                xts.append(parts)

            def bias_copy(engine, dst, src, m):
                """dst = src + bias on the given engine (PSUM -> SBUF).

                Only DVE and ACT can read PSUM on TRN2 (GPSIMD cannot)."""
                if engine == "s":
                    nc.scalar.activation(
                        dst,
                        src,
                        mybir.ActivationFunctionType.Identity,
                        bias=bias_sb[:m, :],
                    )
                else:
                    nc.vector.tensor_scalar_add(dst, src, bias_sb[:m, :])

            nmm = len(terms) * KW

            def do_tile(t):
                r0 = t * M_TILE
                m = min(M_TILE, ROWS_PER_CORE - r0)
                k = m + KH - 1
                # 2 groups of 4 chunks, each on a 4-bank PSUM tile. Matmuls
                # go weight-major (all users of one stationary matrix back
                # to back) and use the full 128-wide band (garbage rows >= m
                # never leave PSUM) so FWL hides the LDWEIGHTS. The two
                # chunk-pairs of each group are copied out in parallel on
                # DVE and ACT into SEPARATE SBUF tiles (write-dependency
                # tracking is tile-granular, so sharing one tile would
                # serialize the copies), then two store DMAs per group on
                # the Sync and Scalar queues.
                for g in range(0, len(chunks), NGRP):
                    grp = chunks[g : g + NGRP]
                    pt = pp.tile([128, NGRP, N_CHUNK], f32, tag="pt")
                    if t == 0 and g == 0:
                        # PE p-state warmup inside the first PSUM tile:
                        # banks are re-initialized by the start=True real
                        # matmuls below
                        for _ in range(10):
                            nc.tensor.matmul(
                                pt[:128, 1, :N_CHUNK],
                                warm[:, :128],
                                warm[:, :],
                                start=True,
                                stop=True,
                            )
                    ndone = [0] * len(grp)
                    for dj in range(KW):
                        for wi in range(nwparts):
                            xis = [xi for wj, xi in terms if wj == wi]
                            lhsT = bandT[:k, wi * KW + dj, :]
                            for xi in xis:
                                for j, (n0, n) in enumerate(grp):
                                    if t == 0:
                                        src = xa0 if g == 0 else xb0
                                        base = XA_LO if g == 0 else XB_LO
                                        rhs = src[xi][
                                            :k, n0 - base + dj : n0 - base + dj + n
                                        ]
                                    else:
                                        rhs = xts[t][xi][:k, n0 + dj : n0 + dj + n]
                                    nc.tensor.matmul(
                                        pt[:128, j, :n],
                                        lhsT,
                                        rhs,
                                        start=(ndone[j] == 0),
                                        stop=(ndone[j] == nmm - 1),
                                    )
                                    ndone[j] += 1
                    goff = grp[0][0]
                    gw = sum(n for _, n in grp)
                    ptf = pt.rearrange("p a b -> p (a b)")
                    half = 2 * N_CHUNK
                    ya = yp.tile([128, half], out_dt, tag="ya")
                    yb = yp.tile([128, half], out_dt, tag="yb")
                    bias_copy("v", ya[:m, :], ptf[:m, :half], m)
                    bias_copy("s", yb[:m, : gw - half], ptf[:m, half:gw], m)
                    nc.sync.dma_start(
                        out=y[r0 : r0 + m, goff : goff + half], in_=ya[:m, :]
                    )
                    nc.scalar.dma_start(
                        out=y[r0 : r0 + m, goff + half : goff + gw],
                        in_=yb[:m, : gw - half],
                    )

            do_tile(0)

            # remainder strip: rows [N_FULL_TILES*M_TILE, ROWS_PER_CORE)
            # for all columns, as NSEG partition-stacked column segments.
            # One 512-wide chunk computes the whole strip; running it right
            # after tile 0 keeps its copy/store off the critical tail.
            ptS = pp.tile([128, NGRP, N_CHUNK], f32, tag="pt")
            ndone = 0
            for dj in range(KW):
                for wi in range(nwparts):
                    xis = [xi for wj, xi in terms if wj == wi]
                    lhsT = bandS[:npart, wi * KW + dj, :]
                    for xi in xis:
                        rhs = strips[xi][:npart, dj : dj + SEG]
                        nc.tensor.matmul(
                            ptS[:128, 0, :SEG],
                            lhsT,
                            rhs,
                            start=(ndone == 0),
                            stop=(ndone == nmm - 1),
                        )
                        ndone += 1
            ys = yp.tile([nout, SEG], out_dt, tag="ystrip")
            nc.vector.tensor_scalar_add(
                ys[:, :], ptS[:nout, 0, :SEG], bias_sb[:nout, :]
            )
            nc.scalar.dma_start(out=ysd[:, :], in_=ys[:, :])

            for t in range(1, n_tiles):
                do_tile(t)
    nc.finalize()
    return nc


def _get_program(variant: str):
    if variant not in _PROGRAM_CACHE:
        _PROGRAM_CACHE[variant] = _build_program(variant)
    return _PROGRAM_CACHE[variant]


def _make_bands(w_parts, bias_val):
    """w_parts: list of [KH, KW] arrays (one per weight split part).

    Returns [128, 2*KW*nw*128 + 1]: flattened full-tile band planes,
    then the block-diagonal strip band planes, then one trailing column
    broadcasting the bias."""
    nw = len(w_parts)
    dtype = w_parts[0].dtype
    # full 128-wide bands: columns >= M_TILE produce garbage output rows
    # that are never copied out of PSUM, but make NumWeights==128 (FWL).
    bt = np.zeros((128, KW * nw, 128), dtype)
    bs = np.zeros((128, KW * nw, 128), dtype)
    for wi, wp in enumerate(w_parts):
        for dj in range(KW):
            for d in range(KH):
                idx = np.arange(128 - d)
                bt[idx + d, wi * KW + dj, idx] = wp[d, dj]
            for blk in range(NSEG):
                for rp in range(R_STRIP):
                    for d in range(KH):
                        bs[
                            STRIP_IN * blk + rp + d,
                            wi * KW + dj,
                            R_STRIP * blk + rp,
                        ] = wp[d, dj]
    nbt = KW * nw * 128
    flat = np.empty((128, 2 * nbt + 1), dtype)
    flat[:, :nbt] = bt.reshape(128, -1)
    flat[:, nbt : 2 * nbt] = bs.reshape(128, -1)
    flat[:, -1] = bias_val
    return flat


def _run(X, weight, bias, trace=False, variant=None):
    from concourse.bass_utils import run_bass_kernel_spmd

    variant = variant or VARIANT
    X = np.ascontiguousarray(np.asarray(X, dtype=np.float32))
    w = np.asarray(weight, dtype=np.float32)
    b = np.asarray(bias, dtype=np.float32)
    assert X.shape == (H, W) and w.shape == (KH, KW)

    nc = _get_program(variant)

    if variant == "f16c":
        Xh = X.astype(np.float16)
        Xl = (X - Xh.astype(np.float32)).astype(np.float16)
        wh = w.astype(np.float16)
        wl = (w - wh.astype(np.float32)).astype(np.float16)
        bands = _make_bands([wh, wl], b[0])
        xparts = [Xh, Xl]
    elif variant in ("f16", "f16o"):
        bands = _make_bands([w.astype(np.float16)], b[0])
        xparts = [X.astype(np.float16)]
    else:
        bands = _make_bands([w], b[0])
        xparts = [X]

    def pack_strip(xp_arr, s):
        rs = s + N_FULL_TILES * M_TILE
        strip = xp_arr[rs : rs + STRIP_IN]  # [10, 4096]
        packed = np.zeros((NSEG * STRIP_IN, SEG + 2), xp_arr.dtype)
        packed[:, :SEG] = (
            strip.reshape(STRIP_IN, NSEG, SEG).transpose(1, 0, 2).reshape(-1, SEG)
        )
        halo = (
            strip[:, SEG:]
            .reshape(STRIP_IN, NSEG - 1, SEG)
            .transpose(1, 0, 2)
            .reshape(-1, SEG)[:, :2]
        )
        packed[: (NSEG - 1) * STRIP_IN, SEG : SEG + 2] = halo
        return packed

    in_maps = []
    for s in STARTS:
        m = {f"x{i}": xp[s : s + IN_ROWS] for i, xp in enumerate(xparts)}
        for i, xp in enumerate(xparts):
            m[f"s{i}"] = pack_strip(xp, s)
        m["bands"] = bands
        in_maps.append(m)
    res = run_bass_kernel_spmd(
        nc, in_maps, core_ids=list(range(NCORES)), trace=trace
    )

    def core_block(c, blk):
        r = res.results[c]
        blk[: N_FULL_TILES * M_TILE] = r["y"][: N_FULL_TILES * M_TILE]
        ys = r["ys"]  # [NSEG*R_STRIP, SEG] packed strip output
        for b_ in range(NSEG):
            wdt = min(SEG, WOUT - b_ * SEG)
            blk[N_FULL_TILES * M_TILE :, b_ * SEG : b_ * SEG + wdt] = ys[
                b_ * R_STRIP : (b_ + 1) * R_STRIP, :wdt
            ]

    out = np.empty((HOUT, WOUT), np.float32)
    for c in range(NCORES - 1):
        core_block(c, out[STARTS[c] : STARTS[c] + ROWS_PER_CORE])
    last = np.empty((ROWS_PER_CORE, WOUT), np.float32)
    core_block(NCORES - 1, last)
    out[STARTS[-1] + 2 :] = last[2:]
    return out, res.exec_time_ns


def kernel(X, weight, bias):
    out, _ = _run(X, weight, bias, trace=False)
    return out
